# revision 74
# baseline (speedup 1.0000x reference)
# Distributed Trainium2 kernel for the GQA attention block
# (nn_Attention_52621939311076).
#
# Sharding: tensor-parallel over heads across 8 NeuronCores. Core c owns
# q-heads [8c, 8c+8) and kv-head c (GQA group stays local). x is replicated,
# wq/wk/wv are sharded on the output dim, wo on the input dim; partial wo
# outputs are summed with an on-device ReduceScatter and the rank slices are
# concatenated on the host.
#
# Everything on device lives in a transposed [feature, seq] layout so that no
# on-chip transposes are needed anywhere:
#   - projections produce Q^T/K^T (head_dim on partitions) and V in [s, d],
#   - RMSNorm reduction over head_dim uses a ones-matmul (partition reduce),
#   - RoPE pairs are (even, odd) partition halves via a host-side permutation
#     of the wq/wk output dims,
#   - attention computes S^T = K^T.T-stationary @ Q^T; softmax denominators
#     are built by accumulating the exp tiles on the Vector engine and doing
#     ONE ones-matmul per (head, q-group) (instead of one per k-block),
#   - O^T = V-stationary @ P^T; the wo matmul consumes O^T directly, and its
#     matmuls are interleaved into the NEXT q-group's attention so the PE
#     never waits on the ACT/DVE softmax chain.
# Matmuls run in bf16 (4x the fp32 TensorE rate), accumulating in fp32 PSUM.
import numpy as np
import ml_dtypes

import concourse.bass as bass
import concourse.bacc as bacc
import concourse.mybir as mybir
import concourse.tile as tile
from concourse.bass_utils import run_bass_kernel_spmd

BF16 = mybir.dt.bfloat16
F32 = mybir.dt.float32
NPBF16 = ml_dtypes.bfloat16

N_CORES = 8
S = 2048          # sequence length
D = 5120          # model dim
H = 64            # q heads (global)
KVH = 8           # kv heads (global)
HD = 128          # head dim
HQ = H // N_CORES  # q heads per core
DC = D // 128     # contraction chunks for the projections
SB = S // 128     # 128-row seq blocks
NG = S // 512     # 512-col seq groups
DG = D // 512     # 512-col output groups for wo
EPS = 1e-6

_cache = {}
RS_CHUNKED = True



def _build(causal: bool):
    nc = bacc.Bacc("TRN2", target_bir_lowering=False, debug=False,
                   num_devices=N_CORES)

    # all big operands are [.., 128, N] with N contiguous per partition, so
    # every load is a 2D DMA with multi-KB descriptor runs
    xt_e = nc.dram_tensor("xt", [NG, 128, DC * 512], BF16,
                          kind="ExternalInput")
    wq_e = nc.dram_tensor("wq", [HQ, 128, DC * 128], BF16,
                          kind="ExternalInput")
    wk_e = nc.dram_tensor("wk", [128, DC * 128], BF16, kind="ExternalInput")
    wv_e = nc.dram_tensor("wv", [128, DC * 128], BF16, kind="ExternalInput")
    wo_e = nc.dram_tensor("wo", [HQ, 128, DG * 512], BF16,
                          kind="ExternalInput")
    cos_e = nc.dram_tensor("cos", [128, S], BF16, kind="ExternalInput")
    sin_e = nc.dram_tensor("sin", [128, S], BF16, kind="ExternalInput")
    qw_e = nc.dram_tensor("qw", [128, 1], F32, kind="ExternalInput")
    kw_e = nc.dram_tensor("kw", [128, 1], F32, kind="ExternalInput")
    if causal:
        # single [k, q] additive mask for the (shared) diagonal 128-block
        mask_e = nc.dram_tensor("mask", [128, 128], F32, kind="ExternalInput")
    else:
        mask_e = nc.dram_tensor("mask", [SB, NG, 128, 512], F32,
                                kind="ExternalInput")
    out_e = nc.dram_tensor("out", [S // N_CORES, D], BF16, kind="ExternalOutput")

    mult = mybir.AluOpType.mult
    Exp = mybir.ActivationFunctionType.Exp
    Sqrt = mybir.ActivationFunctionType.Sqrt
    Square = mybir.ActivationFunctionType.Square

    with tile.TileContext(nc) as tc, \
         tc.tile_pool(name="persist", bufs=1) as persist:
        def single(shape, dtype, name):
            return persist.tile(shape, dtype, name=name, tag=name)

        # ---- persistent SBUF tensors -------------------------------------
        QR = single([128, HQ * S], BF16, "QR")     # roped q, [d, s] per head
        KR = single([128, S], BF16, "KR")          # roped k, [d, s]
        Vsd = single([128, S], BF16, "Vsd")        # v in [s, d], s-block b at cols b*128
        cosT = single([128, S], BF16, "cosT")   # cos duplicated on both halves
        sinT = single([128, S], BF16, "sinT")   # [-sin; +sin]
        qw_t = single([128, 1], F32, "qw_t")
        kw_t = single([128, 1], F32, "kw_t")
        ones_f = single([128, 128], BF16, "ones_f")  # full ones: bcast rowsum
        eps_t = single([128, 1], F32, "eps_t")
        wk_t = single([128, DC * 128], BF16, "wk_t")   # K weights, resident
        wv_t = single([128, DC * 128], BF16, "wv_t")   # V weights, resident
        if causal:
            maskT = single([128, 128], F32, "maskT")

        nc.gpsimd.dma_start(out=cosT[:, :], in_=cos_e[:, :])
        nc.gpsimd.dma_start(out=sinT[:, :], in_=sin_e[:, :])
        nc.gpsimd.dma_start(out=qw_t[:, :], in_=qw_e[:, :])
        nc.gpsimd.dma_start(out=kw_t[:, :], in_=kw_e[:, :])
        nc.vector.memset(ones_f[:, :], 1.0)
        nc.vector.memset(eps_t[:, :], EPS)
        if causal:
            nc.gpsimd.dma_start(out=maskT[:, :], in_=mask_e[:, :])

        # ---- stage 1+2: projections + rmsnorm + rope ---------------------
        def norm_rope(pj, w_ap, dst, dst_cols, sg):
            """pj: PSUM [128,512] projection block; writes roped dst[:, dst_cols]."""
            sq = sqp.tile([128, 512], BF16, tag="sq")
            nc.scalar.activation(sq[:, :], pj[:, :], Square)
            # bc[p, c] = sum_k sq[k, c]  (partition reduce + broadcast in one)
            bc = bcp.tile([128, 512], F32, tag="bc")
            nc.tensor.matmul(bc[:, :], ones_f[:, :], sq[:, :], start=True,
                             stop=True)
            rstd = stats.tile([128, 512], F32, tag="rstd")
            nc.scalar.activation(rstd[:, :], bc[:, :], Sqrt, bias=eps_t[:, :],
                                 scale=1.0 / HD)
            rec = stats.tile([128, 512], F32, tag="rec")
            nc.vector.reciprocal_approx_fast(rec[:, :], rstd[:, :])
            qn = stats.tile([128, 512], BF16, tag="qn")
            # qn = (pj * w) * rec  -- normalized, weighted, cast to bf16
            nc.vector.scalar_tensor_tensor(qn[:, :], pj[:, :], w_ap, rec[:, :],
                                           op0=mult, op1=mult)
            # rope: out = qn*cos2 + swap_halves(qn)*[-sin; sin]; the half-swap
            # is two partition-block SBUF->SBUF DMA copies (no PE involved)
            cs = cosT[:, sg * 512:(sg + 1) * 512]
            sn = sinT[:, sg * 512:(sg + 1) * 512]
            sw = tmps.tile([128, 512], BF16, tag="sw")
            nc.gpsimd.dma_start(out=sw[0:64, :], in_=qn[64:128, :])
            nc.gpsimd.dma_start(out=sw[64:128, :], in_=qn[0:64, :])
            t1 = tmps.tile([128, 512], BF16, tag="t1")
            t2 = tmps.tile([128, 512], BF16, tag="t2")
            nc.vector.tensor_mul(t1[:, :], qn[:, :], cs)
            nc.vector.tensor_mul(t2[:, :], sw[:, :], sn)
            nc.vector.tensor_add(dst[:, dst_cols], t1[:, :], t2[:, :])

        with tc.tile_pool(name="xp", bufs=8) as xp, \
             tc.tile_pool(name="wqp", bufs=2) as wqp, \
             tc.tile_pool(name="sqp", bufs=3) as sqp, \
             tc.tile_pool(name="stats", bufs=4) as stats, \
             tc.tile_pool(name="tmps", bufs=6) as tmps, \
             tc.tile_pool(name="pj", bufs=3, space="PSUM") as pjp, \
             tc.tile_pool(name="bcp", bufs=2, space="PSUM") as bcp, \
             tc.tile_pool(name="pv", bufs=2, space="PSUM") as pvp:
            pending_nr = None
            NS = 5  # x strips per s-group (8 dc each)
            for sg in range(NG):
                # one tile per 8-dc strip; few concurrent DMAs keep each
                # load on its own completion-tracker lane, so projection
                # matmuls chase the DMA arrival front
                xstr = [xp.tile([128, 8 * 512], BF16, name=f"xs{i}",
                                tag="xh") for i in range(NS)]

                def load_strip(i, eng):
                    eng.dma_start(
                        out=xstr[i][:, :],
                        in_=xt_e[sg, :, i * 4096:(i + 1) * 4096])

                if sg == 0:
                    nc.sync.dma_start(out=wk_t[:, :], in_=wk_e[:, :])
                    load_strip(0, nc.scalar)
                    load_strip(1, nc.gpsimd)
                    load_strip(2, nc.sync)
                    load_strip(3, nc.scalar)
                    load_strip(4, nc.gpsimd)
                    nc.sync.dma_start(out=wv_t[:, :], in_=wv_e[:, :])
                else:
                    qs = [nc.sync, nc.scalar]
                    for i in range(NS):
                        load_strip(i, qs[i % 2])

                def xs(dc, c0, w):
                    return xstr[dc // 8][:, (dc % 8) * 512 + c0:
                                         (dc % 8) * 512 + c0 + w]

                cols = slice(sg * 512, (sg + 1) * 512)
                pk = pjp.tile([128, 512], F32, tag="pj")
                for dc in range(DC):
                    nc.tensor.matmul(pk[:, :],
                                     wk_t[:, dc * 128:(dc + 1) * 128],
                                     xs(dc, 0, 512),
                                     start=(dc == 0), stop=(dc == DC - 1))
                # norm_rope is emitted one projection group late so its PE
                # matmul never waits on the ACT/DVE chain of the live group
                if pending_nr is not None:
                    norm_rope(*pending_nr)
                pending_nr = (pk, kw_t[:, :], KR, cols, sg)

                def v_proj(sg=sg, xs=xs):
                    # V in [s, d]: psum [128 s, 128 d] per s-block of group
                    for sb4 in range(4):
                        sb = sg * 4 + sb4
                        pvt = pvp.tile([128, 128], F32, tag="pv")
                        for dc in range(DC):
                            nc.tensor.matmul(
                                pvt[:, :], xs(dc, sb4 * 128, 128),
                                wv_t[:, dc * 128:(dc + 1) * 128],
                                start=(dc == 0), stop=(dc == DC - 1))
                        nc.vector.tensor_copy(Vsd[:, sb * 128:(sb + 1) * 128],
                                              pvt[:, :])

                # first s-group: V after the Q heads (wv lands late on the
                # gpsimd queue); last s-group: V after the Q heads so stage 1
                # ends on a short DVE chain instead of a deep norm_rope chain
                if 0 < sg < NG - 1:
                    v_proj()
                # Q heads
                for qb in range(HQ):
                    wq_t = wqp.tile([128, DC * 128], BF16, tag="wq")
                    nc.sync.dma_start(out=wq_t[:, :], in_=wq_e[qb])
                    pq = pjp.tile([128, 512], F32, tag="pj")
                    for dc in range(DC):
                        nc.tensor.matmul(pq[:, :],
                                         wq_t[:, dc * 128:(dc + 1) * 128],
                                         xs(dc, 0, 512),
                                         start=(dc == 0), stop=(dc == DC - 1))
                    qcols = slice(qb * S + sg * 512, qb * S + (sg + 1) * 512)
                    if pending_nr is not None:
                        norm_rope(*pending_nr)
                    pending_nr = (pq, qw_t[:, :], QR, qcols, sg)
                if sg == NG - 1:
                    norm_rope(*pending_nr)
                    pending_nr = None
                    v_proj()
                elif sg == 0:
                    v_proj()

        # ---- stage 3+4: attention interleaved with wo projection + RS ----
        # Loop s-quarters (q-groups): attention for quarter qg runs with the
        # wo matmuls of quarter qg-1 interleaved between its heads, so the PE
        # has dense work while the ACT/DVE softmax chain drains; each
        # quarter's ReduceScatter then overlaps the following compute.
        with tc.tile_pool(name="ptp", bufs=5) as ptp, \
             tc.tile_pool(name="accp", bufs=3) as accp, \
             tc.tile_pool(name="mgp", bufs=8) as mgp, \
             tc.tile_pool(name="aeps", bufs=3) as aeps, \
             tc.tile_pool(name="otq", bufs=2) as otqp, \
             tc.tile_pool(name="wop", bufs=HQ) as wop, \
             tc.tile_pool(name="oep", bufs=3) as oep, \
             tc.tile_pool(name="stg3p", bufs=5) as stg3p, \
             tc.tile_pool(name="st", bufs=2, space="PSUM") as stp, \
             tc.tile_pool(name="ot", bufs=3, space="PSUM") as otp, \
             tc.tile_pool(name="rs", bufs=1, space="PSUM") as rsp, \
             tc.tile_pool(name="pop", bufs=2, space="PSUM") as pop, \
             tc.tile_pool(name="dram", bufs=1, space="DRAM") as dram:
            # wo weights load dg-major (all heads' slice of one output
            # group per piece) so quarter 0's first wo matmuls wait on
            # ~1 MB, not the whole 10.5 MB
            wos = [wop.tile([128, DG * 512], BF16, name=f"wo{c}", tag="wo")
                   for c in range(HQ)]
            for dg in range(0, DG, 5):
                for c in range(HQ):
                    eng = nc.sync if c % 2 == 0 else nc.gpsimd
                    eng.dma_start(
                        out=wos[c][:, dg * 512:(dg + 5) * 512],
                        in_=wo_e[c, :, dg * 512:(dg + 5) * 512])
            # RS chunks: quarters 0-2 full-quarter ReduceScatters (hidden
            # under later compute); quarter 3 is chunked by 512-column
            # output groups so its RS pipeline drains with the wo matmuls
            # and the serial tail is one small RS.
            chunks = [(0, 0, (0, 4)), (1, 1, (0, 4)), (2, 2, (0, 4))]
            out_base = [0, 64, 128]  # chunk base row in out_e
            partials = [dram.tile([(b1 - b0) * 128, D], BF16,
                                  name=f"partial{i}", tag=f"partial{i}")
                        for i, _, (b0, b1) in chunks]
            rs_outs = [dram.tile([(b1 - b0) * 128 // N_CORES, D], BF16,
                                 name=f"rsout{i}", tag=f"rsout{i}")
                       for i, _, (b0, b1) in chunks]
            # quarter 3: output-column chunks (row-major within the chunk so
            # RS rank-shards align); first/last chunks smaller so the RS
            # stream starts early and ends on a short op
            chunks3 = [(0, 4), (4, 7), (7, 10)]
            partials3 = [dram.tile([512, d1 - d0, 512], BF16,
                                   name=f"p3_{i}", tag=f"p3_{i}")
                         for i, (d0, d1) in enumerate(chunks3)]
            rs3_outs = [dram.tile([512 // N_CORES, d1 - d0, 512], BF16,
                                  name=f"rs3o_{i}", tag=f"rs3o_{i}")
                        for i, (d0, d1) in enumerate(chunks3)]

            def epilogue(h, qg, ot, acc, otq):
                # one ones-matmul turns the DVE-accumulated per-partition
                # partials into broadcast softmax denominators
                rs = rsp.tile([128, 512], F32, tag="rs")
                nc.tensor.matmul(rs[:, :], ones_f[:, :], acc[:, :],
                                 start=True, stop=True)
                rec = aeps.tile([128, 512], F32, tag="arec")
                nc.vector.reciprocal_approx_fast(rec[:, :], rs[:, :])
                nc.vector.tensor_mul(otq[:, h * 512:(h + 1) * 512],
                                     ot[:, :], rec[:, :])

            def wo_stream(qg, otq):
                """Generator emitting quarter qg's (<3) wo matmuls, partials
                DMA, ReduceScatter and out copy; yields after each matmul so
                the emission interleaves with the next quarter's attention."""
                ci = qg
                for sb4 in range(4):
                    for hf in range(2):
                        stg = oep.tile([128, D // 2], BF16, tag="stg")
                        for dg5 in range(5):
                            dg = hf * 5 + dg5
                            po = pop.tile([128, 512], F32, tag="po")
                            for c in range(HQ):
                                nc.tensor.matmul(
                                    po[:, :],
                                    otq[:, c * 512 + sb4 * 128:
                                        c * 512 + (sb4 + 1) * 128],
                                    wos[c][:, dg * 512:(dg + 1) * 512],
                                    start=(c == 0), stop=(c == HQ - 1),
                                    skip_group_check=True)
                                yield
                            nc.vector.tensor_copy(
                                stg[:, dg5 * 512:(dg5 + 1) * 512], po[:, :])
                        # sync+gpsimd: scalar is saturated by the attention
                        # exps, and since the out writes are deferred, the
                        # gpsimd queue holds only non-blocking RS triggers
                        deng = nc.sync if (sb4 + hf) % 2 == 0 else nc.gpsimd
                        deng.dma_start(
                            out=partials[ci][sb4 * 128:(sb4 + 1) * 128,
                                             hf * (D // 2):
                                             (hf + 1) * (D // 2)],
                            in_=stg[:, :])
                        yield
                nc.gpsimd.collective_compute(
                    "ReduceScatter",
                    mybir.AluOpType.add,
                    replica_groups=[list(range(N_CORES))],
                    ins=[partials[ci].opt()],
                    outs=[rs_outs[ci].opt()],
                )
                # the out_e write waits on the RS result, so it must NOT sit
                # in front of any later RS trigger on the gpsimd queue; defer
                # it to after the final trigger
                deferred_outs.append((out_base[ci], rs_outs[ci]))
                yield

            def attention(qg, otq, wo_iter, steps_per_kb):
                nkb = (qg + 1) * 4 if causal else SB
                pending = None  # delayed epilogue: keeps PE off the DVE chain
                for h in range(HQ):
                    qbase = h * S + qg * 512
                    ot = otp.tile([128, 512], F32, tag="ot")
                    acc = accp.tile([128, 512], BF16, tag="acc")
                    for kb in range(nkb):
                        # causal: only q >= kb*128 can attend to this k block
                        c0 = max(0, kb * 128 - qg * 512) if causal else 0
                        st = stp.tile([128, 512], F32, tag="st")
                        nc.tensor.matmul(st[:, c0:],
                                         KR[:, kb * 128:(kb + 1) * 128],
                                         QR[:, qbase + c0:qbase + 512],
                                         start=True, stop=True)
                        if causal:
                            if kb >= qg * 4:  # diagonal block of this q group
                                nc.vector.tensor_add(
                                    st[:, c0:c0 + 128], st[:, c0:c0 + 128],
                                    maskT[:, :])
                        else:
                            mt = mgp.tile([128, 512], F32, tag="mg")
                            nc.sync.dma_start(out=mt[:, :], in_=mask_e[kb, qg])
                            nc.vector.tensor_add(st[:, :], st[:, :], mt[:, :])
                        pt = ptp.tile([128, 512], BF16, tag="pt")
                        nc.scalar.activation(pt[:, c0:], st[:, c0:], Exp)
                        # accumulate the softmax numerator row-sums on DVE
                        # (kb == 0 always has c0 == 0)
                        if kb == 0:
                            nc.vector.tensor_copy(acc[:, :], pt[:, :])
                        else:
                            nc.vector.tensor_add(acc[:, c0:], acc[:, c0:],
                                                 pt[:, c0:])
                        # wo matmuls of the previous quarter slot in here,
                        # while the ACT engine computes this block's exp
                        for _ in range(steps_per_kb):
                            next(wo_iter, None)
                        nc.tensor.matmul(ot[:, c0:],
                                         Vsd[:, kb * 128:(kb + 1) * 128],
                                         pt[:, c0:],
                                         start=(kb == 0), stop=(kb == nkb - 1),
                                         skip_group_check=True)
                    if pending is not None:
                        epilogue(*pending)
                    pending = (h, qg, ot, acc, otq)
                epilogue(*pending)

            otqs = []
            deferred_outs = []
            wo_iter = iter(())
            steps = {0: 0, 1: 6, 2: 4, 3: 5}
            for qg in range(NG):
                otq = otqp.tile([128, HQ * 512], BF16, tag="otq")
                otqs.append(otq)
                attention(qg, otq, wo_iter, steps_per_kb=steps[qg])
                # drain any leftover of the previous quarter's wo stream
                for _ in wo_iter:
                    pass
                if qg < 3:
                    wo_iter = wo_stream(qg, otq)
            # quarter 3: output-column chunks so each chunk's RS
            # fires as soon as its column groups are projected
            otq = otqs[3]
            w3q = 0
            for ci3, (d0, d1) in enumerate(chunks3):
                nd = d1 - d0
                for sb4 in range(4):
                    # stage up to 2 dgs per row-block, written with
                    # contiguous-run DMAs
                    for j in range(0, nd, 2):
                        w = min(2, nd - j)
                        stg = stg3p.tile([128, 2 * 512], BF16, tag="stg3")
                        for dgi in range(w):
                            dg = d0 + j + dgi
                            po = pop.tile([128, 512], F32, tag="po")
                            for c in range(HQ):
                                nc.tensor.matmul(
                                    po[:, :],
                                    otq[:, c * 512 + sb4 * 128:
                                        c * 512 + (sb4 + 1) * 128],
                                    wos[c][:, dg * 512:(dg + 1) * 512],
                                    start=(c == 0), stop=(c == HQ - 1))
                            nc.vector.tensor_copy(
                                stg[:, dgi * 512:(dgi + 1) * 512], po[:, :])
                        deng = nc.sync if w3q % 2 == 0 else nc.scalar
                        w3q += 1
                        deng.dma_start(
                            out=partials3[ci3][sb4 * 128:(sb4 + 1) * 128,
                                               j:j + w, :],
                            in_=stg[:, :w * 512]
                                .rearrange("p (a m) -> p a m", a=w))
                nc.gpsimd.collective_compute(
                    "ReduceScatter",
                    mybir.AluOpType.add,
                    replica_groups=[list(range(N_CORES))],
                    ins=[partials3[ci3].opt()],
                    outs=[rs3_outs[ci3].opt()],
                )
            # all RS triggers are queued; now drain the result writes (each
            # unblocks as its RS completes, in the same order the CC runs)
            for base, rso in deferred_outs:
                nc.gpsimd.dma_start(out=out_e[base:base + 64, :],
                                    in_=rso[:, :])
            for ci3, (d0, d1) in enumerate(chunks3):
                nc.gpsimd.dma_start(
                    out=out_e[192:256, d0 * 512:d1 * 512],
                    in_=rs3_outs[ci3][:, :, :])
    nc.compile()
    return nc


def _host_prep(x, wq, wk, wv, wo, q_norm_w, k_norm_w, freqs_cos, freqs_sin,
               mask, causal):
    xs = x[0]                                    # [S, D] f32
    xt = np.ascontiguousarray(xs.T)              # [D, S]
    # p-major swizzle: [sg, p, dc, m] so each load is contiguous per partition
    xt_t = np.ascontiguousarray(
        xt.reshape(DC, 128, NG, 512).transpose(2, 1, 0, 3)).astype(
            NPBF16).reshape(NG, 128, DC * 512)

    p = np.concatenate([np.arange(0, HD, 2), np.arange(1, HD, 2)])
    c64 = np.ascontiguousarray(freqs_cos.T)                   # [64, S]
    s64 = np.ascontiguousarray(freqs_sin.T)
    cosT = np.concatenate([c64, c64], axis=0).astype(NPBF16)  # [128, S]
    sinT = np.concatenate([-s64, s64], axis=0).astype(NPBF16)

    if causal:
        # all diagonal 128-blocks share the same [k, q] additive mask
        mask_t = np.ascontiguousarray(mask[0:128, 0:128].T).astype(np.float32)
    else:
        mt = np.ascontiguousarray(mask.T)        # [k, q]
        mask_t = np.ascontiguousarray(
            mt.reshape(SB, 128, NG, 512).transpose(0, 2, 1, 3)).astype(np.float32)

    in_maps = []
    for c in range(N_CORES):
        wq_s = wq[c * HQ * HD:(c + 1) * HQ * HD].reshape(HQ, HD, D)[:, p]
        wqT = np.ascontiguousarray(wq_s.reshape(HQ * HD, D).T)   # [D, 1024]
        wq_t = np.ascontiguousarray(
            wqT.reshape(DC, 128, HQ, 128).transpose(2, 1, 0, 3)).astype(
                NPBF16).reshape(HQ, 128, DC * 128)
        wkT = np.ascontiguousarray(wk[c * HD:(c + 1) * HD][p].T)  # [D, 128]
        wk_t = np.ascontiguousarray(
            wkT.reshape(DC, 128, 128).transpose(1, 0, 2)).astype(
                NPBF16).reshape(128, DC * 128)
        wvT = np.ascontiguousarray(wv[c * HD:(c + 1) * HD].T)
        wv_t = np.ascontiguousarray(
            wvT.reshape(DC, 128, 128).transpose(1, 0, 2)).astype(
                NPBF16).reshape(128, DC * 128)
        woT = np.ascontiguousarray(wo[:, c * HQ * HD:(c + 1) * HQ * HD].T)
        wo_t = np.ascontiguousarray(
            woT.reshape(HQ, 128, DG, 512)).astype(
                NPBF16).reshape(HQ, 128, DG * 512)
        qw_v = (q_norm_w[p] / np.sqrt(HD)).astype(np.float32).reshape(HD, 1)
        kw_v = k_norm_w[p].astype(np.float32).reshape(HD, 1)
        in_maps.append({
            "xt": xt_t, "wq": wq_t, "wk": wk_t, "wv": wv_t, "wo": wo_t,
            "cos": cosT, "sin": sinT, "qw": qw_v, "kw": kw_v,
            "mask": mask_t,
        })
    return in_maps


def _numpy_fallback(x, wq, wk, wv, wo, q_norm_w, k_norm_w, cache_k, cache_v,
                    freqs_cos, freqs_sin, mask, start_pos):
    bsz, seqlen, _ = x.shape
    xq = (x @ wq.T).reshape(bsz, seqlen, H, HD)
    xk = (x @ wk.T).reshape(bsz, seqlen, KVH, HD)
    xv = (x @ wv.T).reshape(bsz, seqlen, KVH, HD)

    def rms(v, w):
        n = v * (1.0 / np.sqrt((v * v).mean(-1, keepdims=True) + EPS))
        return n * w

    def rope(v):
        vr = v.reshape(*v.shape[:-1], HD // 2, 2)
        ve, vo = vr[..., 0], vr[..., 1]
        c = freqs_cos[None, :, None, :]
        s = freqs_sin[None, :, None, :]
        oe = ve * c - vo * s
        oo = ve * s + vo * c
        return np.stack([oe, oo], axis=-1).reshape(v.shape)

    xq = rope(rms(xq, q_norm_w))
    xk = rope(rms(xk, k_norm_w))
    ck = np.array(cache_k)
    cv = np.array(cache_v)
    ck[:bsz, start_pos:start_pos + seqlen] = xk
    cv[:bsz, start_pos:start_pos + seqlen] = xv
    kv_len = start_pos + seqlen
    keys = np.repeat(ck[:bsz, :kv_len], H // KVH, axis=2)
    values = np.repeat(cv[:bsz, :kv_len], H // KVH, axis=2)
    sc = np.einsum('bqhd,bkhd->bhqk', xq, keys) / np.sqrt(HD)
    if mask is not None:
        sc = sc + mask[None, None, :, :]
    sc = sc - sc.max(-1, keepdims=True)
    e = np.exp(sc)
    probs = e / e.sum(-1, keepdims=True)
    out = np.einsum('bhqk,bkhd->bqhd', probs, values)
    return (out.reshape(bsz, seqlen, H * HD) @ wo.T).astype(np.float32)


def _run(trace=False, **inputs):
    x = np.asarray(inputs["x"], dtype=np.float32)
    wq = np.asarray(inputs["wq"], dtype=np.float32)
    wk = np.asarray(inputs["wk"], dtype=np.float32)
    wv = np.asarray(inputs["wv"], dtype=np.float32)
    wo = np.asarray(inputs["wo"], dtype=np.float32)
    q_norm_w = np.asarray(inputs["q_norm_w"], dtype=np.float32)
    k_norm_w = np.asarray(inputs["k_norm_w"], dtype=np.float32)
    freqs_cos = np.asarray(inputs["freqs_cos"], dtype=np.float32)
    freqs_sin = np.asarray(inputs["freqs_sin"], dtype=np.float32)
    mask = np.asarray(inputs["mask"], dtype=np.float32)
    start_pos = int(inputs.get("start_pos", 0))

    if start_pos != 0 or x.shape != (1, S, D):
        return _numpy_fallback(
            x, wq, wk, wv, wo, q_norm_w, k_norm_w,
            np.asarray(inputs["cache_k"]), np.asarray(inputs["cache_v"]),
            freqs_cos, freqs_sin, mask, start_pos), None

    causal = bool(
        (mask == np.triu(np.full((S, S), -1e9, dtype=np.float32), k=1)).all())

    key = ("nc", causal)
    if key not in _cache:
        _cache[key] = _build(causal)
    nc = _cache[key]
    in_maps = _host_prep(x, wq, wk, wv, wo, q_norm_w, k_norm_w,
                         freqs_cos, freqs_sin, mask, causal)
    res = run_bass_kernel_spmd(nc, in_maps, core_ids=list(range(N_CORES)),
                               trace=trace)
    out = np.empty((S, D), dtype=np.float32)
    chunks = [(0, 0, 0, 64), (1, 512, 64, 64), (2, 1024, 128, 64),
              (3, 1536, 192, 64)]
    for r in range(N_CORES):
        o = np.asarray(res.results[r]["out"], dtype=np.float32)
        if RS_CHUNKED:
            for ci, gbase, obase, rows in chunks:
                out[gbase + r * rows:gbase + (r + 1) * rows] = \
                    o[obase:obase + rows]
        else:
            out[r * 256:(r + 1) * 256] = o
    return out.reshape(1, S, D), res


def kernel(**inputs) -> np.ndarray:
    out, _ = _run(trace=False, **inputs)
    return out


# revision 75
# speedup vs baseline: 1.0900x; 1.0900x over previous
# Distributed Trainium2 kernel for the GQA attention block
# (nn_Attention_52621939311076).
#
# Sharding: tensor-parallel over heads across 8 NeuronCores. Core c owns
# q-heads [8c, 8c+8) and kv-head c (GQA group stays local). x is replicated,
# wq/wk/wv are sharded on the output dim, wo on the input dim; partial wo
# outputs are summed with an on-device ReduceScatter and the rank slices are
# concatenated on the host.
#
# Everything on device lives in a transposed [feature, seq] layout so that no
# on-chip transposes are needed anywhere:
#   - projections produce Q^T/K^T (head_dim on partitions) and V in [s, d],
#   - RMSNorm reduction over head_dim uses a ones-matmul (partition reduce),
#   - RoPE pairs are (even, odd) partition halves via a host-side permutation
#     of the wq/wk output dims,
#   - attention computes S^T = K^T.T-stationary @ Q^T; softmax denominators
#     are built by accumulating the exp tiles on the Vector engine and doing
#     ONE ones-matmul per (head, q-group) (instead of one per k-block),
#   - O^T = V-stationary @ P^T; the wo matmul consumes O^T directly, and its
#     matmuls are interleaved into the NEXT q-group's attention so the PE
#     never waits on the ACT/DVE softmax chain.
# Matmuls run in bf16 (4x the fp32 TensorE rate), accumulating in fp32 PSUM.
import numpy as np
import ml_dtypes

import concourse.bass as bass
import concourse.bacc as bacc
import concourse.mybir as mybir
import concourse.tile as tile
from concourse.bass_utils import run_bass_kernel_spmd

BF16 = mybir.dt.bfloat16
F32 = mybir.dt.float32
NPBF16 = ml_dtypes.bfloat16

N_CORES = 8
S = 2048          # sequence length
D = 5120          # model dim
H = 64            # q heads (global)
KVH = 8           # kv heads (global)
HD = 128          # head dim
HQ = H // N_CORES  # q heads per core
DC = D // 128     # contraction chunks for the projections
SB = S // 128     # 128-row seq blocks
NG = S // 512     # 512-col seq groups
DG = D // 512     # 512-col output groups for wo
EPS = 1e-6

_cache = {}
RS_CHUNKED = True



def _build(causal: bool):
    nc = bacc.Bacc("TRN2", target_bir_lowering=False, debug=False,
                   num_devices=N_CORES)

    # all big operands are [.., 128, N] with N contiguous per partition, so
    # every load is a 2D DMA with multi-KB descriptor runs
    xt_e = nc.dram_tensor("xt", [NG, 128, DC * 512], BF16,
                          kind="ExternalInput")
    wq_e = nc.dram_tensor("wq", [HQ, 128, DC * 128], BF16,
                          kind="ExternalInput")
    wk_e = nc.dram_tensor("wk", [128, DC * 128], BF16, kind="ExternalInput")
    wv_e = nc.dram_tensor("wv", [128, DC * 128], BF16, kind="ExternalInput")
    wo_e = nc.dram_tensor("wo", [HQ, 128, DG * 512], BF16,
                          kind="ExternalInput")
    cos_e = nc.dram_tensor("cos", [128, S], BF16, kind="ExternalInput")
    sin_e = nc.dram_tensor("sin", [128, S], BF16, kind="ExternalInput")
    qw_e = nc.dram_tensor("qw", [128, 1], F32, kind="ExternalInput")
    kw_e = nc.dram_tensor("kw", [128, 1], F32, kind="ExternalInput")
    if causal:
        # single [k, q] additive mask for the (shared) diagonal 128-block
        mask_e = nc.dram_tensor("mask", [128, 128], F32, kind="ExternalInput")
    else:
        mask_e = nc.dram_tensor("mask", [SB, NG, 128, 512], F32,
                                kind="ExternalInput")
    out_e = nc.dram_tensor("out", [S // N_CORES, D], BF16, kind="ExternalOutput")

    mult = mybir.AluOpType.mult
    Exp = mybir.ActivationFunctionType.Exp
    Sqrt = mybir.ActivationFunctionType.Sqrt
    Square = mybir.ActivationFunctionType.Square

    with tile.TileContext(nc) as tc, \
         tc.tile_pool(name="persist", bufs=1) as persist:
        def single(shape, dtype, name):
            return persist.tile(shape, dtype, name=name, tag=name)

        # ---- persistent SBUF tensors -------------------------------------
        QR = single([128, HQ * S], BF16, "QR")     # roped q, [d, s] per head
        KR = single([128, S], BF16, "KR")          # roped k, [d, s]
        Vsd = single([128, S], BF16, "Vsd")        # v in [s, d], s-block b at cols b*128
        cosT = single([128, S], BF16, "cosT")   # cos duplicated on both halves
        sinT = single([128, S], BF16, "sinT")   # [-sin; +sin]
        qw_t = single([128, 1], F32, "qw_t")
        kw_t = single([128, 1], F32, "kw_t")
        ones_f = single([128, 128], BF16, "ones_f")  # full ones: bcast rowsum
        eps_t = single([128, 1], F32, "eps_t")
        wk_t = single([128, DC * 128], BF16, "wk_t")   # K weights, resident
        wv_t = single([128, DC * 128], BF16, "wv_t")   # V weights, resident
        if causal:
            maskT = single([128, 128], F32, "maskT")

        nc.gpsimd.dma_start(out=cosT[:, :], in_=cos_e[:, :])
        nc.gpsimd.dma_start(out=sinT[:, :], in_=sin_e[:, :])
        nc.gpsimd.dma_start(out=qw_t[:, :], in_=qw_e[:, :])
        nc.gpsimd.dma_start(out=kw_t[:, :], in_=kw_e[:, :])
        nc.vector.memset(ones_f[:, :], 1.0)
        nc.vector.memset(eps_t[:, :], EPS)
        if causal:
            nc.gpsimd.dma_start(out=maskT[:, :], in_=mask_e[:, :])

        # ---- stage 1+2: projections + rmsnorm + rope ---------------------
        def norm_rope(pj, w_ap, dst, dst_cols, sg):
            """pj: PSUM [128,512] projection block; writes roped dst[:, dst_cols]."""
            sq = sqp.tile([128, 512], BF16, tag="sq")
            nc.scalar.activation(sq[:, :], pj[:, :], Square)
            # bc[p, c] = sum_k sq[k, c]  (partition reduce + broadcast in one)
            bc = bcp.tile([128, 512], F32, tag="bc")
            nc.tensor.matmul(bc[:, :], ones_f[:, :], sq[:, :], start=True,
                             stop=True)
            rstd = stats.tile([128, 512], F32, tag="rstd")
            nc.scalar.activation(rstd[:, :], bc[:, :], Sqrt, bias=eps_t[:, :],
                                 scale=1.0 / HD)
            rec = stats.tile([128, 512], F32, tag="rec")
            nc.vector.reciprocal_approx_fast(rec[:, :], rstd[:, :])
            qn = stats.tile([128, 512], BF16, tag="qn")
            # qn = (pj * w) * rec  -- normalized, weighted, cast to bf16
            nc.vector.scalar_tensor_tensor(qn[:, :], pj[:, :], w_ap, rec[:, :],
                                           op0=mult, op1=mult)
            # rope: out = qn*cos2 + swap_halves(qn)*[-sin; sin]; the half-swap
            # is two partition-block SBUF->SBUF DMA copies (no PE involved)
            cs = cosT[:, sg * 512:(sg + 1) * 512]
            sn = sinT[:, sg * 512:(sg + 1) * 512]
            sw = tmps.tile([128, 512], BF16, tag="sw")
            nc.gpsimd.dma_start(out=sw[0:64, :], in_=qn[64:128, :])
            nc.gpsimd.dma_start(out=sw[64:128, :], in_=qn[0:64, :])
            t1 = tmps.tile([128, 512], BF16, tag="t1")
            t2 = tmps.tile([128, 512], BF16, tag="t2")
            nc.vector.tensor_mul(t1[:, :], qn[:, :], cs)
            nc.vector.tensor_mul(t2[:, :], sw[:, :], sn)
            nc.vector.tensor_add(dst[:, dst_cols], t1[:, :], t2[:, :])

        with tc.tile_pool(name="xp", bufs=8) as xp, \
             tc.tile_pool(name="wqp", bufs=2) as wqp, \
             tc.tile_pool(name="sqp", bufs=3) as sqp, \
             tc.tile_pool(name="stats", bufs=4) as stats, \
             tc.tile_pool(name="tmps", bufs=6) as tmps, \
             tc.tile_pool(name="pj", bufs=3, space="PSUM") as pjp, \
             tc.tile_pool(name="bcp", bufs=2, space="PSUM") as bcp, \
             tc.tile_pool(name="pv", bufs=2, space="PSUM") as pvp:
            pending_nr = None
            NS = 5  # x strips per s-group (8 dc each)
            for sg in range(NG):
                # one tile per 8-dc strip; few concurrent DMAs keep each
                # load on its own completion-tracker lane, so projection
                # matmuls chase the DMA arrival front
                xstr = [xp.tile([128, 8 * 512], BF16, name=f"xs{i}",
                                tag="xh") for i in range(NS)]

                def load_strip(i, eng):
                    eng.dma_start(
                        out=xstr[i][:, :],
                        in_=xt_e[sg, :, i * 4096:(i + 1) * 4096])

                if sg == 0:
                    nc.sync.dma_start(out=wk_t[:, :], in_=wk_e[:, :])
                    load_strip(0, nc.scalar)
                    load_strip(1, nc.gpsimd)
                    load_strip(2, nc.sync)
                    load_strip(3, nc.scalar)
                    load_strip(4, nc.gpsimd)
                    nc.sync.dma_start(out=wv_t[:, :], in_=wv_e[:, :])
                else:
                    qs = [nc.sync, nc.scalar]
                    for i in range(NS):
                        load_strip(i, qs[i % 2])

                def xs(dc, c0, w):
                    return xstr[dc // 8][:, (dc % 8) * 512 + c0:
                                         (dc % 8) * 512 + c0 + w]

                cols = slice(sg * 512, (sg + 1) * 512)
                pk = pjp.tile([128, 512], F32, tag="pj")
                for dc in range(DC):
                    nc.tensor.matmul(pk[:, :],
                                     wk_t[:, dc * 128:(dc + 1) * 128],
                                     xs(dc, 0, 512),
                                     start=(dc == 0), stop=(dc == DC - 1))
                # norm_rope is emitted one projection group late so its PE
                # matmul never waits on the ACT/DVE chain of the live group
                if pending_nr is not None:
                    norm_rope(*pending_nr)
                pending_nr = (pk, kw_t[:, :], KR, cols, sg)

                def v_proj(sg=sg, xs=xs):
                    # V in [s, d]: psum [128 s, 128 d] per s-block of group
                    for sb4 in range(4):
                        sb = sg * 4 + sb4
                        pvt = pvp.tile([128, 128], F32, tag="pv")
                        for dc in range(DC):
                            nc.tensor.matmul(
                                pvt[:, :], xs(dc, sb4 * 128, 128),
                                wv_t[:, dc * 128:(dc + 1) * 128],
                                start=(dc == 0), stop=(dc == DC - 1))
                        nc.vector.tensor_copy(Vsd[:, sb * 128:(sb + 1) * 128],
                                              pvt[:, :])

                # first s-group: V after the Q heads (wv lands late on the
                # gpsimd queue); last s-group: V after the Q heads so stage 1
                # ends on a short DVE chain instead of a deep norm_rope chain
                if 0 < sg < NG - 1:
                    v_proj()
                # Q heads
                for qb in range(HQ):
                    wq_t = wqp.tile([128, DC * 128], BF16, tag="wq")
                    nc.sync.dma_start(out=wq_t[:, :], in_=wq_e[qb])
                    pq = pjp.tile([128, 512], F32, tag="pj")
                    for dc in range(DC):
                        nc.tensor.matmul(pq[:, :],
                                         wq_t[:, dc * 128:(dc + 1) * 128],
                                         xs(dc, 0, 512),
                                         start=(dc == 0), stop=(dc == DC - 1))
                    qcols = slice(qb * S + sg * 512, qb * S + (sg + 1) * 512)
                    if pending_nr is not None:
                        norm_rope(*pending_nr)
                    pending_nr = (pq, qw_t[:, :], QR, qcols, sg)
                if sg == NG - 1:
                    norm_rope(*pending_nr)
                    pending_nr = None
                    v_proj()
                elif sg == 0:
                    v_proj()

        # ---- stage 3+4: attention interleaved with wo projection + RS ----
        # Loop s-quarters (q-groups): attention for quarter qg runs with the
        # wo matmuls of quarter qg-1 interleaved between its heads, so the PE
        # has dense work while the ACT/DVE softmax chain drains; each
        # quarter's ReduceScatter then overlaps the following compute.
        with tc.tile_pool(name="ptp", bufs=5) as ptp, \
             tc.tile_pool(name="accp", bufs=3) as accp, \
             tc.tile_pool(name="mgp", bufs=8) as mgp, \
             tc.tile_pool(name="aeps", bufs=3) as aeps, \
             tc.tile_pool(name="otq", bufs=2) as otqp, \
             tc.tile_pool(name="wop", bufs=HQ) as wop, \
             tc.tile_pool(name="oep", bufs=3) as oep, \
             tc.tile_pool(name="stg3p", bufs=5) as stg3p, \
             tc.tile_pool(name="st", bufs=2, space="PSUM") as stp, \
             tc.tile_pool(name="ot", bufs=3, space="PSUM") as otp, \
             tc.tile_pool(name="rs", bufs=1, space="PSUM") as rsp, \
             tc.tile_pool(name="pop", bufs=2, space="PSUM") as pop, \
             tc.tile_pool(name="dram", bufs=1, space="DRAM") as dram:
            # wo weights load dg-major (all heads' slice of one output
            # group per piece) so quarter 0's first wo matmuls wait on
            # ~1 MB, not the whole 10.5 MB
            wos = [wop.tile([128, DG * 512], BF16, name=f"wo{c}", tag="wo")
                   for c in range(HQ)]
            for dg in range(0, DG, 5):
                for c in range(HQ):
                    eng = nc.sync if c % 2 == 0 else nc.gpsimd
                    eng.dma_start(
                        out=wos[c][:, dg * 512:(dg + 5) * 512],
                        in_=wo_e[c, :, dg * 512:(dg + 5) * 512])
            # RS chunks: quarters 0-2 full-quarter ReduceScatters (hidden
            # under later compute); quarter 3 is chunked by 512-column
            # output groups so its RS pipeline drains with the wo matmuls
            # and the serial tail is one small RS.
            chunks = [(0, 0, (0, 4)), (1, 1, (0, 4)), (2, 2, (0, 4))]
            out_base = [0, 64, 128]  # chunk base row in out_e
            partials = [dram.tile([(b1 - b0) * 128, D], BF16,
                                  name=f"partial{i}", tag=f"partial{i}")
                        for i, _, (b0, b1) in chunks]
            rs_outs = [dram.tile([(b1 - b0) * 128 // N_CORES, D], BF16,
                                 name=f"rsout{i}", tag=f"rsout{i}")
                       for i, _, (b0, b1) in chunks]
            # quarter 3: output-column chunks (row-major within the chunk so
            # RS rank-shards align); first/last chunks smaller so the RS
            # stream starts early and ends on a short op
            chunks3 = [(0, 4), (4, 7), (7, 10)]
            partials3 = [dram.tile([512, d1 - d0, 512], BF16,
                                   name=f"p3_{i}", tag=f"p3_{i}")
                         for i, (d0, d1) in enumerate(chunks3)]
            rs3_outs = [dram.tile([512 // N_CORES, d1 - d0, 512], BF16,
                                  name=f"rs3o_{i}", tag=f"rs3o_{i}")
                        for i, (d0, d1) in enumerate(chunks3)]

            def epilogue(h, qg, ot, acc, otq):
                # one ones-matmul turns the DVE-accumulated per-partition
                # partials into broadcast softmax denominators
                rs = rsp.tile([128, 512], F32, tag="rs")
                nc.tensor.matmul(rs[:, :], ones_f[:, :], acc[:, :],
                                 start=True, stop=True)
                rec = aeps.tile([128, 512], F32, tag="arec")
                nc.vector.reciprocal_approx_fast(rec[:, :], rs[:, :])
                nc.vector.tensor_mul(otq[:, h * 512:(h + 1) * 512],
                                     ot[:, :], rec[:, :])

            def wo_stream(qg, otq):
                """Generator emitting quarter qg's (<3) wo matmuls, partials
                DMA, ReduceScatter and out copy; yields after each matmul so
                the emission interleaves with the next quarter's attention."""
                ci = qg
                for sb4 in range(4):
                    for hf in range(2):
                        stg = oep.tile([128, D // 2], BF16, tag="stg")
                        for dg5 in range(5):
                            dg = hf * 5 + dg5
                            po = pop.tile([128, 512], F32, tag="po")
                            for c in range(HQ):
                                nc.tensor.matmul(
                                    po[:, :],
                                    otq[:, c * 512 + sb4 * 128:
                                        c * 512 + (sb4 + 1) * 128],
                                    wos[c][:, dg * 512:(dg + 1) * 512],
                                    start=(c == 0), stop=(c == HQ - 1),
                                    skip_group_check=True)
                                yield
                            nc.vector.tensor_copy(
                                stg[:, dg5 * 512:(dg5 + 1) * 512], po[:, :])
                        # keep partial writes off the gpsimd queue: they must
                        # not sit behind a collective dispatch
                        deng = nc.sync if (sb4 + hf) % 2 == 0 else nc.scalar
                        deng.dma_start(
                            out=partials[ci][sb4 * 128:(sb4 + 1) * 128,
                                             hf * (D // 2):
                                             (hf + 1) * (D // 2)],
                            in_=stg[:, :])
                        yield
                nc.gpsimd.collective_compute(
                    "ReduceScatter",
                    mybir.AluOpType.add,
                    replica_groups=[list(range(N_CORES))],
                    ins=[partials[ci].opt()],
                    outs=[rs_outs[ci].opt()],
                )
                # the out_e write waits on the RS result, so it must NOT sit
                # in front of any later RS trigger on the gpsimd queue; defer
                # it to after the final trigger
                deferred_outs.append((out_base[ci], rs_outs[ci]))
                yield

            def attention(qg, otq, wo_iter, steps_per_kb):
                nkb = (qg + 1) * 4 if causal else SB
                pending = None  # delayed epilogue: keeps PE off the DVE chain
                for h in range(HQ):
                    qbase = h * S + qg * 512
                    ot = otp.tile([128, 512], F32, tag="ot")
                    acc = accp.tile([128, 512], BF16, tag="acc")
                    for kb in range(nkb):
                        # causal: only q >= kb*128 can attend to this k block
                        c0 = max(0, kb * 128 - qg * 512) if causal else 0
                        st = stp.tile([128, 512], F32, tag="st")
                        nc.tensor.matmul(st[:, c0:],
                                         KR[:, kb * 128:(kb + 1) * 128],
                                         QR[:, qbase + c0:qbase + 512],
                                         start=True, stop=True)
                        if causal:
                            if kb >= qg * 4:  # diagonal block of this q group
                                nc.vector.tensor_add(
                                    st[:, c0:c0 + 128], st[:, c0:c0 + 128],
                                    maskT[:, :])
                        else:
                            mt = mgp.tile([128, 512], F32, tag="mg")
                            nc.sync.dma_start(out=mt[:, :], in_=mask_e[kb, qg])
                            nc.vector.tensor_add(st[:, :], st[:, :], mt[:, :])
                        pt = ptp.tile([128, 512], BF16, tag="pt")
                        nc.scalar.activation(pt[:, c0:], st[:, c0:], Exp)
                        # accumulate the softmax numerator row-sums on DVE
                        # (kb == 0 always has c0 == 0)
                        if kb == 0:
                            nc.vector.tensor_copy(acc[:, :], pt[:, :])
                        else:
                            nc.vector.tensor_add(acc[:, c0:], acc[:, c0:],
                                                 pt[:, c0:])
                        # wo matmuls of the previous quarter slot in here,
                        # while the ACT engine computes this block's exp
                        for _ in range(steps_per_kb):
                            next(wo_iter, None)
                        nc.tensor.matmul(ot[:, c0:],
                                         Vsd[:, kb * 128:(kb + 1) * 128],
                                         pt[:, c0:],
                                         start=(kb == 0), stop=(kb == nkb - 1),
                                         skip_group_check=True)
                    if pending is not None:
                        epilogue(*pending)
                    pending = (h, qg, ot, acc, otq)
                epilogue(*pending)

            otqs = []
            deferred_outs = []
            wo_iter = iter(())
            steps = {0: 0, 1: 6, 2: 4, 3: 5}
            for qg in range(NG):
                otq = otqp.tile([128, HQ * 512], BF16, tag="otq")
                otqs.append(otq)
                attention(qg, otq, wo_iter, steps_per_kb=steps[qg])
                # drain any leftover of the previous quarter's wo stream
                for _ in wo_iter:
                    pass
                if qg < 3:
                    wo_iter = wo_stream(qg, otq)
            # quarter 3: output-column chunks so each chunk's RS
            # fires as soon as its column groups are projected
            otq = otqs[3]
            w3q = 0
            for ci3, (d0, d1) in enumerate(chunks3):
                nd = d1 - d0
                for sb4 in range(4):
                    # stage up to 2 dgs per row-block, written with
                    # contiguous-run DMAs
                    for j in range(0, nd, 2):
                        w = min(2, nd - j)
                        stg = stg3p.tile([128, 2 * 512], BF16, tag="stg3")
                        for dgi in range(w):
                            dg = d0 + j + dgi
                            po = pop.tile([128, 512], F32, tag="po")
                            for c in range(HQ):
                                nc.tensor.matmul(
                                    po[:, :],
                                    otq[:, c * 512 + sb4 * 128:
                                        c * 512 + (sb4 + 1) * 128],
                                    wos[c][:, dg * 512:(dg + 1) * 512],
                                    start=(c == 0), stop=(c == HQ - 1))
                            nc.vector.tensor_copy(
                                stg[:, dgi * 512:(dgi + 1) * 512], po[:, :])
                        deng = nc.sync if w3q % 2 == 0 else nc.scalar
                        w3q += 1
                        deng.dma_start(
                            out=partials3[ci3][sb4 * 128:(sb4 + 1) * 128,
                                               j:j + w, :],
                            in_=stg[:, :w * 512]
                                .rearrange("p (a m) -> p a m", a=w))
                nc.gpsimd.collective_compute(
                    "ReduceScatter",
                    mybir.AluOpType.add,
                    replica_groups=[list(range(N_CORES))],
                    ins=[partials3[ci3].opt()],
                    outs=[rs3_outs[ci3].opt()],
                )
            # all RS triggers are queued; now drain the result writes (each
            # unblocks as its RS completes, in the same order the CC runs)
            for base, rso in deferred_outs:
                nc.gpsimd.dma_start(out=out_e[base:base + 64, :],
                                    in_=rso[:, :])
            for ci3, (d0, d1) in enumerate(chunks3):
                nc.gpsimd.dma_start(
                    out=out_e[192:256, d0 * 512:d1 * 512],
                    in_=rs3_outs[ci3][:, :, :])
    nc.compile()
    return nc


def _host_prep(x, wq, wk, wv, wo, q_norm_w, k_norm_w, freqs_cos, freqs_sin,
               mask, causal):
    xs = x[0]                                    # [S, D] f32
    xt = np.ascontiguousarray(xs.T)              # [D, S]
    # p-major swizzle: [sg, p, dc, m] so each load is contiguous per partition
    xt_t = np.ascontiguousarray(
        xt.reshape(DC, 128, NG, 512).transpose(2, 1, 0, 3)).astype(
            NPBF16).reshape(NG, 128, DC * 512)

    p = np.concatenate([np.arange(0, HD, 2), np.arange(1, HD, 2)])
    c64 = np.ascontiguousarray(freqs_cos.T)                   # [64, S]
    s64 = np.ascontiguousarray(freqs_sin.T)
    cosT = np.concatenate([c64, c64], axis=0).astype(NPBF16)  # [128, S]
    sinT = np.concatenate([-s64, s64], axis=0).astype(NPBF16)

    if causal:
        # all diagonal 128-blocks share the same [k, q] additive mask
        mask_t = np.ascontiguousarray(mask[0:128, 0:128].T).astype(np.float32)
    else:
        mt = np.ascontiguousarray(mask.T)        # [k, q]
        mask_t = np.ascontiguousarray(
            mt.reshape(SB, 128, NG, 512).transpose(0, 2, 1, 3)).astype(np.float32)

    in_maps = []
    for c in range(N_CORES):
        wq_s = wq[c * HQ * HD:(c + 1) * HQ * HD].reshape(HQ, HD, D)[:, p]
        wqT = np.ascontiguousarray(wq_s.reshape(HQ * HD, D).T)   # [D, 1024]
        wq_t = np.ascontiguousarray(
            wqT.reshape(DC, 128, HQ, 128).transpose(2, 1, 0, 3)).astype(
                NPBF16).reshape(HQ, 128, DC * 128)
        wkT = np.ascontiguousarray(wk[c * HD:(c + 1) * HD][p].T)  # [D, 128]
        wk_t = np.ascontiguousarray(
            wkT.reshape(DC, 128, 128).transpose(1, 0, 2)).astype(
                NPBF16).reshape(128, DC * 128)
        wvT = np.ascontiguousarray(wv[c * HD:(c + 1) * HD].T)
        wv_t = np.ascontiguousarray(
            wvT.reshape(DC, 128, 128).transpose(1, 0, 2)).astype(
                NPBF16).reshape(128, DC * 128)
        woT = np.ascontiguousarray(wo[:, c * HQ * HD:(c + 1) * HQ * HD].T)
        wo_t = np.ascontiguousarray(
            woT.reshape(HQ, 128, DG, 512)).astype(
                NPBF16).reshape(HQ, 128, DG * 512)
        qw_v = (q_norm_w[p] / np.sqrt(HD)).astype(np.float32).reshape(HD, 1)
        kw_v = k_norm_w[p].astype(np.float32).reshape(HD, 1)
        in_maps.append({
            "xt": xt_t, "wq": wq_t, "wk": wk_t, "wv": wv_t, "wo": wo_t,
            "cos": cosT, "sin": sinT, "qw": qw_v, "kw": kw_v,
            "mask": mask_t,
        })
    return in_maps


def _numpy_fallback(x, wq, wk, wv, wo, q_norm_w, k_norm_w, cache_k, cache_v,
                    freqs_cos, freqs_sin, mask, start_pos):
    bsz, seqlen, _ = x.shape
    xq = (x @ wq.T).reshape(bsz, seqlen, H, HD)
    xk = (x @ wk.T).reshape(bsz, seqlen, KVH, HD)
    xv = (x @ wv.T).reshape(bsz, seqlen, KVH, HD)

    def rms(v, w):
        n = v * (1.0 / np.sqrt((v * v).mean(-1, keepdims=True) + EPS))
        return n * w

    def rope(v):
        vr = v.reshape(*v.shape[:-1], HD // 2, 2)
        ve, vo = vr[..., 0], vr[..., 1]
        c = freqs_cos[None, :, None, :]
        s = freqs_sin[None, :, None, :]
        oe = ve * c - vo * s
        oo = ve * s + vo * c
        return np.stack([oe, oo], axis=-1).reshape(v.shape)

    xq = rope(rms(xq, q_norm_w))
    xk = rope(rms(xk, k_norm_w))
    ck = np.array(cache_k)
    cv = np.array(cache_v)
    ck[:bsz, start_pos:start_pos + seqlen] = xk
    cv[:bsz, start_pos:start_pos + seqlen] = xv
    kv_len = start_pos + seqlen
    keys = np.repeat(ck[:bsz, :kv_len], H // KVH, axis=2)
    values = np.repeat(cv[:bsz, :kv_len], H // KVH, axis=2)
    sc = np.einsum('bqhd,bkhd->bhqk', xq, keys) / np.sqrt(HD)
    if mask is not None:
        sc = sc + mask[None, None, :, :]
    sc = sc - sc.max(-1, keepdims=True)
    e = np.exp(sc)
    probs = e / e.sum(-1, keepdims=True)
    out = np.einsum('bhqk,bkhd->bqhd', probs, values)
    return (out.reshape(bsz, seqlen, H * HD) @ wo.T).astype(np.float32)


def _run(trace=False, **inputs):
    x = np.asarray(inputs["x"], dtype=np.float32)
    wq = np.asarray(inputs["wq"], dtype=np.float32)
    wk = np.asarray(inputs["wk"], dtype=np.float32)
    wv = np.asarray(inputs["wv"], dtype=np.float32)
    wo = np.asarray(inputs["wo"], dtype=np.float32)
    q_norm_w = np.asarray(inputs["q_norm_w"], dtype=np.float32)
    k_norm_w = np.asarray(inputs["k_norm_w"], dtype=np.float32)
    freqs_cos = np.asarray(inputs["freqs_cos"], dtype=np.float32)
    freqs_sin = np.asarray(inputs["freqs_sin"], dtype=np.float32)
    mask = np.asarray(inputs["mask"], dtype=np.float32)
    start_pos = int(inputs.get("start_pos", 0))

    if start_pos != 0 or x.shape != (1, S, D):
        return _numpy_fallback(
            x, wq, wk, wv, wo, q_norm_w, k_norm_w,
            np.asarray(inputs["cache_k"]), np.asarray(inputs["cache_v"]),
            freqs_cos, freqs_sin, mask, start_pos), None

    causal = bool(
        (mask == np.triu(np.full((S, S), -1e9, dtype=np.float32), k=1)).all())

    key = ("nc", causal)
    if key not in _cache:
        _cache[key] = _build(causal)
    nc = _cache[key]
    in_maps = _host_prep(x, wq, wk, wv, wo, q_norm_w, k_norm_w,
                         freqs_cos, freqs_sin, mask, causal)
    res = run_bass_kernel_spmd(nc, in_maps, core_ids=list(range(N_CORES)),
                               trace=trace)
    out = np.empty((S, D), dtype=np.float32)
    chunks = [(0, 0, 0, 64), (1, 512, 64, 64), (2, 1024, 128, 64),
              (3, 1536, 192, 64)]
    for r in range(N_CORES):
        o = np.asarray(res.results[r]["out"], dtype=np.float32)
        if RS_CHUNKED:
            for ci, gbase, obase, rows in chunks:
                out[gbase + r * rows:gbase + (r + 1) * rows] = \
                    o[obase:obase + rows]
        else:
            out[r * 256:(r + 1) * 256] = o
    return out.reshape(1, S, D), res


def kernel(**inputs) -> np.ndarray:
    out, _ = _run(trace=False, **inputs)
    return out
